# revision 28
# baseline (speedup 1.0000x reference)
"""Trainium2 Bass/Tile kernel for nn_CausalSelfAttention (relu-attention).

Sharding: 8 cores = batch B(4) x head-group(2 groups of 6 heads).
Each core computes, for (batch b, head group g):
  qT,kT = (W_{q,k}^g)^T x_b^T   (transposed layout, [d_cols, T])
  v     = x_b W_v^g             (natural layout,   [T, d_cols])
  per head h in group:
     attT[k,q] = relu( kT_h^T(tile) . qT_h ) * causal    (k on partitions)
     [out_rawT; S] = [v*m | m]^T . attT                  (augmented matmul)
     outT_h = out_rawT * (1/(S+1e-9)) broadcast
  partial = outT^T W_p^g        -> host sums the two group partials per batch.

The (1 + lambda*log(q+1)) row scale cancels in the relu-attention
normalization (scale >= 1 > 0), up to the 1e-9 epsilon; verified
numerically to ~1e-7 rel err, so layer_lambda is unused.

Matmuls use float32r (TF32-like, full PE rate at free-dim>=256).
"""

from contextlib import ExitStack

import numpy as np

import concourse.bass as bass
import concourse.mybir as mybir
import concourse.tile as tile
from concourse import bacc
from concourse.bass_utils import run_bass_kernel_spmd

B, T, C, H = 4, 2048, 768, 12
D = C // H            # 64 head dim
HG = H // 2           # 6 heads per group
GC = HG * D           # 384 cols per head group
P = 128
NCT = C // P          # 6 input-channel tiles
NT = T // P           # 16 token tiles
CH = 512              # q-chunk width
NQC = T // CH         # 4 q chunks
f32 = mybir.dt.float32
f32r = mybir.dt.float32r
EPS = 1e-9

_NC_CACHE = None


def _emit(ctx, tc, xT, wq, wk, wv, wp, msk, out, dbg=None):
    nc = tc.nc
    Relu = mybir.ActivationFunctionType.Relu
    ctr = [0]

    def relu_copy(dst, src):
        # 2:1 toward ScalarE — VectorE carries masks/norm/vaug work
        if ctr[0] % 3 != 2:
            nc.scalar.activation(dst, src, Relu)
        else:
            nc.vector.tensor_scalar_max(dst, src, 0.0)
        ctr[0] += 1

    def copy(dst, src):
        if ctr[0] % 2 == 0:
            nc.scalar.copy(dst, src)
        else:
            nc.vector.tensor_copy(dst, src)
        ctr[0] += 1

    const = ctx.enter_context(tc.tile_pool(name="const", bufs=1))
    persist = ctx.enter_context(tc.tile_pool(name="persist", bufs=1))

    # mask tiled [p, tile] and replicated [p, h, tile]
    mt = const.tile([P, NT], f32, tag="mt", name="mt")
    nc.sync.dma_start(out=mt, in_=msk.rearrange("(n p) -> p n", p=P))
    mrep = const.tile([P, HG, NT], f32, tag="mrep", name="mrep")
    for hh in range(HG):
        nc.sync.dma_start(out=mrep[:, hh, :], in_=msk.rearrange("(n p) -> p n", p=P))

    # causal masks for diagonal blocks of attT[k_part, q_free]:
    # keep where q >= k.  mask128[p, f] = (f - p >= 0)
    ones = const.tile([P, 256], f32, tag="ones", name="ones")
    nc.vector.memset(ones, 1.0)
    mask128 = const.tile([P, 128], f32, tag="m128", name="m128")
    nc.gpsimd.affine_select(out=mask128, in_=ones[:, 0:128],
                            compare_op=mybir.AluOpType.is_ge, fill=0.0,
                            base=0, pattern=[[1, 128]], channel_multiplier=-1)
    mask256 = const.tile([P, 256], f32, tag="m256", name="m256")
    nc.gpsimd.affine_select(out=mask256, in_=ones,
                            compare_op=mybir.AluOpType.is_ge, fill=0.0,
                            base=-128, pattern=[[1, 256]], channel_multiplier=-1)

    # persistent tensors
    qts = [persist.tile([P, T], f32, tag=f"qt{i}", name=f"qt{i}") for i in range(3)]
    kts = [persist.tile([P, T], f32, tag=f"kt{i}", name=f"kt{i}") for i in range(3)]
    vaugs = [persist.tile([P, HG, D + 1], f32, tag=f"va{i}", name=f"va{i}") for i in range(NT)]
    outT = [persist.tile([P, T], f32, tag=f"ot{i}", name=f"ot{i}") for i in range(3)]
    wps = [persist.tile([P, C], f32, tag=f"wp{i}", name=f"wp{i}") for i in range(3)]
    for i in range(3):
        nc.sync.dma_start(out=wps[i].bitcast(f32r), in_=wp[i * P:(i + 1) * P, :].bitcast(f32r))

    # ---------- Phase A: qT, kT (transposed) and v-augmented (natural) ----
    with tc.tile_pool(name="phA", bufs=1) as phA, \
         tc.tile_pool(name="psA", bufs=3, space="PSUM") as psA, \
         tc.tile_pool(name="psV", bufs=3, space="PSUM") as psV:
        xts = [phA.tile([P, T], f32, tag=f"xt{i}", name=f"xt{i}") for i in range(NCT)]
        wqs = [phA.tile([P, GC], f32, tag=f"wq{i}", name=f"wq{i}") for i in range(NCT)]
        wks = [phA.tile([P, GC], f32, tag=f"wk{i}", name=f"wk{i}") for i in range(NCT)]
        wvs = [phA.tile([P, GC], f32, tag=f"wv{i}", name=f"wv{i}") for i in range(NCT)]
        # order: (wq,x) pairs first so the first qT matmul group can start
        # as early as possible, then wk, then wv
        for i in range(NCT):
            sl = slice(i * P, (i + 1) * P)
            nc.sync.dma_start(out=wqs[i].bitcast(f32r), in_=wq[sl, :].bitcast(f32r))
            nc.sync.dma_start(out=xts[i].bitcast(f32r), in_=xT[sl, :].bitcast(f32r))
        for i in range(NCT):
            sl = slice(i * P, (i + 1) * P)
            nc.sync.dma_start(out=wks[i].bitcast(f32r), in_=wk[sl, :].bitcast(f32r))
        for i in range(NCT):
            sl = slice(i * P, (i + 1) * P)
            nc.sync.dma_start(out=wvs[i].bitcast(f32r), in_=wv[sl, :].bitcast(f32r))

        # qT/kT: [col_tile(128), t] = sum_c W[c_tile, col_tile]^T @ xT[c_tile, t]
        for dst, ws in ((qts, wqs), (kts, wks)):
            for ct in range(3):
                for ch in range(NQC):
                    ps = psA.tile([P, CH], f32, tag="psqk", name="psqk")
                    for c in range(NCT):
                        nc.tensor.matmul(
                            ps,
                            lhsT=ws[c][:, ct * P:(ct + 1) * P].bitcast(f32r),
                            rhs=xts[c][:, ch * CH:(ch + 1) * CH].bitcast(f32r),
                            start=(c == 0), stop=(c == NCT - 1))
                    copy(dst[ct][:, ch * CH:(ch + 1) * CH].bitcast(f32r), ps)

        # v natural + augmentation with mask column
        for tt in range(NT):
            ps = psV.tile([P, GC], f32, tag="psv", name="psv")
            for c in range(NCT):
                nc.tensor.matmul(
                    ps,
                    lhsT=xts[c][:, tt * P:(tt + 1) * P].bitcast(f32r),
                    rhs=wvs[c].bitcast(f32r),
                    start=(c == 0), stop=(c == NCT - 1))
            ps3 = ps.rearrange("p (h d) -> p h d", h=HG)
            nc.vector.tensor_scalar_mul(vaugs[tt][:, :, 0:D].bitcast(f32r), ps3,
                                        mt[:, tt:tt + 1])
            nc.vector.tensor_copy(vaugs[tt][:, :, D].bitcast(f32r), mrep[:, :, tt])

    # ---------- Phase B: attention per head ------------------------------
    with tc.tile_pool(name="attp", bufs=2) as attp, \
         tc.tile_pool(name="normp", bufs=2) as normp, \
         tc.tile_pool(name="psSC", bufs=3, space="PSUM") as psSC, \
         tc.tile_pool(name="psAV", bufs=1, space="PSUM") as psAV:
        for h in range(HG):
            rt = h // 2
            ro = (h % 2) * D
            qt_h = qts[rt][ro:ro + D, :]
            kt_h = kts[rt][ro:ro + D, :]
            av_ps = psAV.tile([D + 1, T], f32, tag="av", name="av")
            for j in range(NT):
                c0 = j // 4
                start = P * j if j % 4 != 3 else P * (j - 1)
                att = attp.tile([P, T], f32, tag="att", name="att")
                # scores for region [start, T) of q, relu'd into att
                for c in range(c0, NQC):
                    cs = max(start, CH * c)
                    ce = CH * (c + 1)
                    ps = psSC.tile([P, CH], f32, tag="sc", name="sc")
                    nc.tensor.matmul(
                        ps[:, 0:ce - cs],
                        lhsT=kt_h[:, j * P:(j + 1) * P].bitcast(f32r),
                        rhs=qt_h[:, cs:ce].bitcast(f32r),
                        start=True, stop=True)
                    if c == c0:
                        # causal mask on the diagonal block, on PSUM pre-relu
                        mw = 256 if j % 4 == 3 else 128
                        msk_t = mask256 if j % 4 == 3 else mask128
                        nc.vector.tensor_mul(ps[:, 0:mw], ps[:, 0:mw], msk_t)
                    relu_copy(att[:, cs:ce].bitcast(f32r), ps[:, 0:ce - cs])
                # accumulate [v*m | m]^T @ attT
                for c in range(c0, NQC):
                    cs = max(start, CH * c)
                    ce = CH * (c + 1)
                    nc.tensor.matmul(
                        av_ps[:, cs:ce],
                        lhsT=vaugs[j][:, h, :].bitcast(f32r),
                        rhs=att[:, cs:ce].bitcast(f32r),
                        start=(j == 0), stop=(j == 4 * c + 3))
            # evacuate av_ps to SBUF fast (split ACT/DVE) to release the
            # PSUM accumulator for the next head, then normalize off SBUF
            raw = normp.tile([D + 1, T], f32, tag="raw", name="raw", bufs=3)
            nc.scalar.copy(raw[:, 0:T // 2], av_ps[:, 0:T // 2])
            nc.vector.tensor_copy(raw[:, T // 2:T], av_ps[:, T // 2:T])
            # S row into its own offset-0 tile: partition_broadcast only
            # honors a partition-0 source on HW
            s_sb = normp.tile([1, T], f32, tag="s", name="s")
            nc.scalar.copy(s_sb, raw[D:D + 1, :])
            s_bc = normp.tile([D, T], f32, tag="sbc", name="sbc")
            nc.gpsimd.partition_broadcast(out_ap=s_bc, in_ap=s_sb)
            nc.vector.tensor_scalar_add(s_bc, s_bc, EPS)
            rec = normp.tile([D, T], f32, tag="rec", name="rec")
            nc.vector.reciprocal(rec, s_bc)
            nc.vector.tensor_mul(outT[rt][ro:ro + D, :].bitcast(f32r), raw[0:D, :], rec)
            if dbg is not None and h == 0:
                nc.sync.dma_start(out=dbg["raw0"], in_=raw)
                nc.sync.dma_start(out=dbg["rec0"], in_=rec)
        if dbg is not None:
            nc.sync.dma_start(out=dbg["qt0"], in_=qts[0])
            nc.sync.dma_start(out=dbg["kt0"], in_=kts[0])
            nc.sync.dma_start(out=dbg["va0"], in_=vaugs[0].rearrange("p h d -> p (h d)"))
            nc.sync.dma_start(out=dbg["va5"], in_=vaugs[5].rearrange("p h d -> p (h d)"))
            nc.sync.dma_start(out=dbg["ot0"], in_=outT[0])

    # ---------- Phase C: projection --------------------------------------
    with tc.tile_pool(name="outp", bufs=3) as outp, \
         tc.tile_pool(name="psP", bufs=2, space="PSUM") as psP:
        for tt in range(NT):
            ps1 = psP.tile([P, CH], f32, tag="p1", name="p1")
            ps2 = psP.tile([P, C - CH], f32, tag="p2", name="p2")
            for r in range(3):
                lt = outT[r][:, tt * P:(tt + 1) * P].bitcast(f32r)
                nc.tensor.matmul(ps1, lhsT=lt, rhs=wps[r][:, 0:CH].bitcast(f32r),
                                 start=(r == 0), stop=(r == 2))
                nc.tensor.matmul(ps2, lhsT=lt, rhs=wps[r][:, CH:C].bitcast(f32r),
                                 start=(r == 0), stop=(r == 2))
            ob = outp.tile([P, C], f32, tag="ob", name="ob")
            copy(ob[:, 0:CH], ps1)
            copy(ob[:, CH:C], ps2)
            nc.sync.dma_start(out=out[tt * P:(tt + 1) * P, :], in_=ob)


def build_nc(debug=False, reps=1):
    nc = bacc.Bacc("TRN2", target_bir_lowering=False, debug=False,
                   enable_asserts=False, num_devices=8)
    xT = nc.dram_tensor("xT", [C, T], f32, kind="ExternalInput").ap()
    wq = nc.dram_tensor("wq", [C, GC], f32, kind="ExternalInput").ap()
    wk = nc.dram_tensor("wk", [C, GC], f32, kind="ExternalInput").ap()
    wv = nc.dram_tensor("wv", [C, GC], f32, kind="ExternalInput").ap()
    wp = nc.dram_tensor("wp", [GC, C], f32, kind="ExternalInput").ap()
    msk = nc.dram_tensor("msk", [T], f32, kind="ExternalInput").ap()
    out = nc.dram_tensor("out", [T, C], f32, kind="ExternalOutput").ap()
    dbg = None
    if debug:
        dbg = {
            "raw0": nc.dram_tensor("raw0", [D + 1, T], f32, kind="ExternalOutput").ap(),
            "rec0": nc.dram_tensor("rec0", [D, T], f32, kind="ExternalOutput").ap(),
            "qt0": nc.dram_tensor("qt0", [P, T], f32, kind="ExternalOutput").ap(),
            "kt0": nc.dram_tensor("kt0", [P, T], f32, kind="ExternalOutput").ap(),
            "va0": nc.dram_tensor("va0", [P, HG * (D + 1)], f32, kind="ExternalOutput").ap(),
            "va5": nc.dram_tensor("va5", [P, HG * (D + 1)], f32, kind="ExternalOutput").ap(),
            "ot0": nc.dram_tensor("ot0", [P, T], f32, kind="ExternalOutput").ap(),
        }
    with tile.TileContext(nc) as tc:
        for _ in range(reps):
            with ExitStack() as ctx:
                _emit(ctx, tc, xT, wq, wk, wv, wp, msk, out, dbg=dbg)
    nc.compile()
    return nc


def make_in_maps(inputs):
    x = np.ascontiguousarray(np.asarray(inputs["x"], dtype=np.float32))
    attn_mask = np.asarray(inputs["attn_mask"], dtype=np.float32)
    W_attn = np.asarray(inputs["W_attn"], dtype=np.float32)
    W_proj = np.asarray(inputs["W_proj"], dtype=np.float32)
    in_maps = []
    for core in range(8):
        b, g = divmod(core, 2)
        cs = slice(g * GC, (g + 1) * GC)
        in_maps.append({
            "xT": np.ascontiguousarray(x[b].T),
            "wq": np.ascontiguousarray(W_attn[:, 0 * C:1 * C][:, cs]),
            "wk": np.ascontiguousarray(W_attn[:, 1 * C:2 * C][:, cs]),
            "wv": np.ascontiguousarray(W_attn[:, 2 * C:3 * C][:, cs]),
            "wp": np.ascontiguousarray(W_proj[cs, :]),
            "msk": np.ascontiguousarray(attn_mask[b]),
        })
    return in_maps


def combine_outs(outs):
    full = np.empty((B, T, C), dtype=np.float32)
    for b in range(B):
        full[b] = outs[2 * b] + outs[2 * b + 1]
    return full


def kernel(**inputs):
    global _NC_CACHE
    if _NC_CACHE is None:
        _NC_CACHE = build_nc()
    nc = _NC_CACHE
    res = run_bass_kernel_spmd(nc, make_in_maps(inputs), list(range(8)))
    return combine_outs([res.results[i]["out"] for i in range(8)])


def make_runner(nc, n_cores=8):
    """Persistent-jit SPMD runner (mirrors bass2jax.run_bass_via_pjrt's
    multi-core branch) so repeated calls reuse the compiled executable —
    needed for wall-clock timing since this axon client has no NTFF hook."""
    import jax
    from jax.experimental.shard_map import shard_map
    from jax.sharding import Mesh, PartitionSpec
    from concourse import bass2jax
    import concourse.mybir as _mybir

    bass2jax.install_neuronx_cc_hook()
    partition_name = nc.partition_id_tensor.name if nc.partition_id_tensor else None
    in_names, out_names, out_avals, zero_outs = [], [], [], []
    for alloc in nc.m.functions[0].allocations:
        if not isinstance(alloc, _mybir.MemoryLocationSet):
            continue
        name = alloc.memorylocations[0].name
        if alloc.kind == "ExternalInput":
            if name != partition_name:
                in_names.append(name)
        elif alloc.kind == "ExternalOutput":
            shape = tuple(alloc.tensor_shape)
            dtype = _mybir.dt.np(alloc.dtype)
            out_names.append(name)
            out_avals.append(jax.core.ShapedArray(shape, dtype))
            zero_outs.append(np.zeros(shape, dtype))
    n_params = len(in_names)
    n_outs = len(out_avals)
    all_in_names = list(in_names) + list(out_names)
    if partition_name is not None:
        all_in_names.append(partition_name)

    def _body(*args):
        operands = list(args)
        if partition_name is not None:
            operands.append(bass2jax.partition_id_tensor())
        return tuple(bass2jax._bass_exec_p.bind(
            *operands, out_avals=tuple(out_avals), in_names=tuple(all_in_names),
            out_names=tuple(out_names), lowering_input_output_aliases=(),
            sim_require_finite=True, sim_require_nnan=True, nc=nc))

    devices = jax.devices()[:n_cores]
    mesh = Mesh(np.asarray(devices), ("core",))
    sharded = jax.jit(
        shard_map(_body, mesh=mesh,
                  in_specs=(PartitionSpec("core"),) * (n_params + n_outs),
                  out_specs=(PartitionSpec("core"),) * n_outs,
                  check_rep=False),
        donate_argnums=tuple(range(n_params, n_params + n_outs)),
        keep_unused=True)

    def run(in_maps):
        per_core = [[np.asarray(m[name]) for name in in_names] for m in in_maps]
        concat_in = [np.concatenate([per_core[c][i] for c in range(n_cores)], axis=0)
                     for i in range(n_params)]
        concat_zeros = [np.zeros((n_cores * z.shape[0], *z.shape[1:]), z.dtype)
                        for z in zero_outs]
        out_arrs = sharded(*concat_in, *concat_zeros)
        jax.block_until_ready(out_arrs)
        return [
            {name: np.asarray(out_arrs[i]).reshape(n_cores, *out_avals[i].shape)[c]
             for i, name in enumerate(out_names)}
            for c in range(n_cores)
        ]

    return run
